# revision 70
# baseline (speedup 1.0000x reference)
"""DifferentialAttention (B=2, S=2048, D=2048, H=16, KVH=8) on 8 TRN2 NeuronCores.

Sharding: 8 cores = 2 (batch) x 4 (tensor-parallel head groups).
Core c = 4*b + r handles batch b and real heads 4r..4r+3.

Design (cost-model-driven; ~294us vs 359us for the all-bf16 version):
  - q/k/v and o projections run as fp8e4m3 hi+lo DoubleRow matmuls: weights
    and x are split host-side (x on-chip for otf) into a high fp8 part and an
    fp8 residual; the three cross terms hi*hi, lo*hi, hi*lo are computed as
    k-tile-paired DoubleRow matmuls at 0.5 cyc/col (lo*lo dropped, ~0.1%),
    i.e. 0.75x the bf16 PE cost at bf16-level accuracy. Power-of-2 scales
    (w x64, x x4, otf x4 via the rsqrt bias) fold exactly into the rope
    tables (2^-8), the exp scale (1/8 = Dh^-0.5), the v copy (2^-8) and a
    host-side 2^-8 on the output. Naked fp8 (single-part) was measured at
    2.7e-2..5.4e-2 final error -- over the 2e-2 gate -- so scores/AV stay
    bf16 (fp8 hi/lo has no win there: the K=64 contraction already halves
    the array, and exp output can't be split into hi/lo cheaply).
  - all weights resident in SBUF; weight DMAs issued from the Pool/ACT
    queues, x from SP, so streams overlap and nothing gates the first matmul
  - RoPE rotate-half via ONE stream_shuffle (head dims host-permuted so the
    rotate partner sits +/-16 within the same 32-partition quadrant); the
    sin-multiply runs on Pool for pumped shards, DVE for direct shards
  - scores for both doubled heads -> one 2-bank psum tile [128,1024]; ONE
    merged exp per ki; AV/rowsum consumption lags the exp by TWO ki so the
    ACT->PE handoff is off the critical path
  - causal mask as a 0/1 triu multiply on the E tile (Pool), in the lag slack
  - softmax denominators via skinny N=1 single-shot matmuls into half-bank
    psum slots (manually double-buffered) + one DVE reduce; per-q scalars on
    [128,4] tiles; partition broadcast via PE transpose + bf16 selector
    matmuls
  - RMS rsqrt per qi-group on DVE: bit-hack + 2 Newton steps on [128,16]
    (keeps ACT exp-only, no act-table switches); norm folded as
    u * rsqrt(pre * 2^-11) with pre = sum(u^2) + 128*eps*R1^2
  - o_proj reads fp8 otf hi/lo (hi written by the normalize mul on DVE, lo
    residual by a Pool subtract) against resident fp8 wo hi/lo
  - phase interleave: projection shards 2,3, o_proj groups 0..2-half and the
    bfive applies are thunks pumped between attention ki iterations; the
    last o_proj blocks fill the bfive_group(3) chains at the tail
  - GPSIMD ops never touch PSUM (hardware restriction the cost model does
    not check)
"""

import math
import numpy as np
import ml_dtypes

B, S, D = 2, 2048, 2048
H, KVH = 16, 8
Dh = 64
TP = 4
NCORES = 8
LAYER_IDX = 2
LAMBDA_INIT = 0.8 - 0.6 * math.exp(-0.3 * LAYER_IDX)
EPS = 1e-5
ROPE_THETA = 10000.0

_CACHE = {}


def _build_nc():
    import concourse.bass as bass  # noqa: F401
    import concourse.tile as tile
    from concourse import bacc, mybir

    F32 = mybir.dt.float32
    F32R = mybir.dt.float32r
    BF16 = mybir.dt.bfloat16
    F8 = mybir.dt.float8e4
    DR = mybir.MatmulPerfMode.DoubleRow
    Act = mybir.ActivationFunctionType
    Alu = mybir.AluOpType

    nc = bacc.Bacc("TRN2", target_bir_lowering=False, debug=False)

    # fp8 hi|lo pairs: x scaled x4, weights x64; descale folded into the
    # rope cos/sin tables (2^-8), the exp activation scale (1/8 = the Dh^-1/2
    # score scaling), and the v psum->sbuf copy (2^-8)
    xT = nc.dram_tensor("xT", [D, 2 * S], F8, kind="ExternalInput")
    wqT = nc.dram_tensor("wqT", [D, 1024], F8, kind="ExternalInput")
    wkT = nc.dram_tensor("wkT", [D, 512], F8, kind="ExternalInput")
    wvT = nc.dram_tensor("wvT", [D, 512], F8, kind="ExternalInput")
    woT = nc.dram_tensor("woT", [512, 2 * D], F8, kind="ExternalInput")
    cosT_d = nc.dram_tensor("cosT", [128, S], BF16, kind="ExternalInput")
    ssinT_d = nc.dram_tensor("ssinT", [128, S], BF16, kind="ExternalInput")
    maskT_d = nc.dram_tensor("maskT", [128, 128], BF16, kind="ExternalInput")
    ident_d = nc.dram_tensor("ident", [128, 128], F32, kind="ExternalInput")
    onescol_d = nc.dram_tensor("onescol", [128, 1], BF16, kind="ExternalInput")
    sel4_d = nc.dram_tensor("sel4", [4, 512], BF16, kind="ExternalInput")
    lam_d = nc.dram_tensor("lam", [128, 1], F32, kind="ExternalInput")
    out_d = nc.dram_tensor("out", [S, D], BF16, kind="ExternalOutput")

    KD = D // 128
    SHUF = [(i + 16) % 32 for i in range(32)]
    SQ128E = 128.0 * EPS

    with tile.TileContext(nc) as tc:
        with tc.tile_pool(name="const", bufs=1) as constp, \
             tc.tile_pool(name="persist", bufs=1) as persist, \
             tc.tile_pool(name="xtp", bufs=8) as xtp, \
             tc.tile_pool(name="ropet", bufs=3) as rp, \
             tc.tile_pool(name="etp", bufs=4) as etp, \
             tc.tile_pool(name="ebp", bufs=2) as ebp, \
             tc.tile_pool(name="outp", bufs=4) as outp, \
             tc.tile_pool(name="psS", bufs=2, space="PSUM") as psS, \
             tc.tile_pool(name="psOT", bufs=1, space="PSUM") as psOT, \
             tc.tile_pool(name="psSm", bufs=1, space="PSUM") as psSm, \
             tc.tile_pool(name="psBG", bufs=1, space="PSUM") as psBG:

            cosT = constp.tile([128, S], BF16, tag="cos")
            ssinT = constp.tile([128, S], BF16, tag="ssin")
            maskT = constp.tile([128, 128], BF16, tag="mask")
            ident = constp.tile([128, 128], F32, tag="id")
            onescol = constp.tile([128, 1], BF16, tag="onc")
            sel4 = constp.tile([4, 512], BF16, tag="sel4")
            lam = constp.tile([128, 1], F32, tag="lam")

            wq_sb = persist.tile([128, KD * 1024], F8, tag="wq")
            wk_sb = persist.tile([128, KD * 512], F8, tag="wk")
            wv_sb = persist.tile([128, KD * 512], F8, tag="wv")
            wo_sb = persist.tile([128, 4 * 2 * 2048], F8, tag="wo")

            qT_sb = [persist.tile([128, S], BF16, tag=f"qT{m}", name=f"qT{m}")
                     for m in range(4)]
            kTd = [persist.tile([128, S], BF16, tag=f"kTd{p}", name=f"kTd{p}")
                   for p in range(4)]
            v_sb = [persist.tile([128, 256], BF16, tag=f"v{ms}", name=f"v{ms}")
                    for ms in range(16)]
            otf = [persist.tile([128, S], BF16, tag=f"otf{p}", name=f"otf{p}")
                   for p in range(4)]
            otf8 = persist.tile([128, 2 * 4 * S], F8, tag="otf8")
            otf8v = otf8[:].rearrange("p (two kc s) -> p two kc s",
                                      two=2, kc=4)
            pre_all = persist.tile([128, 64], F32, tag="pre")
            sf_all = persist.tile([128, 64], F32, tag="sf")

            wqv = wq_sb[:].rearrange("p (kd two n) -> p kd two n", kd=KD, two=2)
            wkv = wk_sb[:].rearrange("p (kd two n) -> p kd two n", kd=KD, two=2)
            wvv = wv_sb[:].rearrange("p (kd two n) -> p kd two n", kd=KD, two=2)
            wov = wo_sb[:].rearrange("p (kc two n) -> p kc two n",
                                     kc=4, two=2)

            smalls_all = psSm.tile([128, 512], F32, tag="sm", name="smalls")

            xt_tiles = {}

            def load_x_one(sh, kp, eng=None):
                c0 = 512 * sh
                t = xtp.tile([128, 4096], F8, tag="xt", name=f"xt{sh}_{kp}")
                tv = t[:].rearrange("p (two four n) -> p two four n",
                                    two=2, four=4)
                src = xT[kp * 512:kp * 512 + 512, :] \
                    .rearrange("(four p) (two s) -> p two four s",
                               four=4, two=2)
                (eng or nc.sync).dma_start(
                    out=tv, in_=src[:, :, :, c0:c0 + 512])
                xt_tiles[sh, kp] = t

            def load_x(sh):
                for kp in range(4):
                    load_x_one(sh, kp)

            def xt_pair(sh, p2, hl, msl=None):
                # k-tile pair (kd=2*p2, 2*p2+1) of the hi (hl=0) / lo (hl=1)
                # component; msl slices the 512-wide seq cols
                v = xt_tiles[sh, p2 // 2][:] \
                    .rearrange("p (two four n) -> p two four n", two=2, four=4)
                lp = (p2 % 2) * 2
                if msl is None:
                    return v[:, hl, lp:lp + 2, :]
                return v[:, hl, lp:lp + 2, msl]

            # x on the SP queue; weights on the ACT queue -- parallel DMA
            # streams
            load_x(0)
            load_x(1)
            nc.gpsimd.dma_start(
                out=wqv[:, 0:2, :, :],
                in_=wqT[0:256, :].rearrange(
                    "(kd p) (two n) -> p kd two n", kd=2, two=2))
            nc.gpsimd.dma_start(
                out=wqv[:, 2:4, :, :],
                in_=wqT[256:512, :].rearrange(
                    "(kd p) (two n) -> p kd two n", kd=2, two=2))
            for qt in range(1, 4):
                nc.gpsimd.dma_start(
                    out=wqv[:, qt * 4:(qt + 1) * 4, :, :],
                    in_=wqT[qt * 512:(qt + 1) * 512, :]
                        .rearrange("(kd p) (two n) -> p kd two n",
                                   kd=4, two=2),
                )
            nc.scalar.dma_start(out=cosT[:], in_=cosT_d[:])
            nc.scalar.dma_start(out=ssinT[:], in_=ssinT_d[:])
            nc.gpsimd.dma_start(
                out=wkv[:],
                in_=wkT[:].rearrange("(kd p) (two n) -> p kd two n",
                                     kd=KD, two=2))
            nc.gpsimd.dma_start(
                out=wvv[:],
                in_=wvT[:].rearrange("(kd p) (two n) -> p kd two n",
                                     kd=KD, two=2))
            nc.scalar.dma_start(out=maskT[:], in_=maskT_d[:])
            nc.scalar.dma_start(out=ident[:], in_=ident_d[:])
            nc.scalar.dma_start(out=onescol[:], in_=onescol_d[:])
            nc.scalar.dma_start(out=sel4[:], in_=sel4_d[:])
            nc.scalar.dma_start(out=lam[:], in_=lam_d[:])
            nc.scalar.dma_start(
                out=wov[:],
                in_=woT[:].rearrange("(kc p) (two n) -> p kc two n",
                                     kc=4, two=2))

            # ============ projections (direct or pumped thunks) ============
            def rope_q(m, ps, csl, mule):
                qsw = rp.tile([128, 512], F32, tag="sw", name="qsw")
                nc.vector.stream_shuffle(qsw[:], ps[:, 0:512], SHUF)
                qc = rp.tile([128, 512], F32, tag="qc", name="qc")
                nc.vector.tensor_mul(qc[:], ps[:, 0:512], cosT[:, csl])
                mule.tensor_mul(qsw[:], qsw[:], ssinT[:, csl])
                nc.vector.tensor_add(qT_sb[m][:, csl], qc[:], qsw[:])

            def rope_k(m, ps, csl, mule):
                ksw = rp.tile([128, 512], F32, tag="sw", name="ksw")
                nc.vector.stream_shuffle(ksw[:], ps[:, 0:512], SHUF)
                kc = rp.tile([128, 512], F32, tag="qc", name="kc")
                nc.vector.tensor_mul(kc[:], ps[:, 0:512], cosT[:, csl])
                mule.tensor_mul(ksw[:], ksw[:], ssinT[:, csl])
                for e in range(2):
                    esl = slice(e * 64, e * 64 + 64)
                    for hf in range(2):
                        nc.vector.tensor_add(
                            kTd[2 * m + e][hf * 64:hf * 64 + 64, csl],
                            kc[esl, :], ksw[esl, :])

            def proj_tiles(sh, pool):
                c0 = 512 * sh
                csl = slice(c0, c0 + 512)
                out = []
                alt = [0]
                for kind, m in ([("q", m) for m in range(4)]
                                + [("k", m) for m in range(2)]
                                + [("v", ms) for ms in range(4)]):
                    box = {}
                    if isinstance(pool, list):
                        upool = pool[alt[0] % len(pool)]
                        alt[0] += 1
                    else:
                        upool = pool

                    def mmgrp(g, kind=kind, m=m, box=box, upool=upool):
                        def f():
                            if g == 0:
                                if upool is psBG:
                                    box["ps"] = upool.tile([128, 512], F32,
                                                           tag="bg", name="bgp")
                                else:
                                    box["ps"] = upool.tile([128, 1024], F32,
                                                           tag="s", name="prp")
                            ps = box["ps"]
                            msl = slice(m * 128, m * 128 + 128)
                            # 3-term hi/lo fp8 DoubleRow per kd pair:
                            #   (w_hi,w_hi)x(x_hi,x_hi) + (w_lo,w_lo)x(x_hi,
                            #   x_hi) + (w_hi,w_hi)x(x_lo,x_lo); lo*lo dropped
                            for p2 in (2 * g, 2 * g + 1):
                                ks = slice(2 * p2, 2 * p2 + 2)
                                st = (p2 == 0)
                                sp = (p2 == 7)
                                if kind in ("q", "k"):
                                    wv_ = wqv if kind == "q" else wkv
                                    for i, (wc, xc) in enumerate(
                                            ((0, 0), (1, 0), (0, 1))):
                                        nc.tensor.matmul(
                                            ps[:, 0:512],
                                            wv_[:, ks, wc, msl],
                                            xt_pair(sh, p2, xc),
                                            start=st and i == 0,
                                            stop=sp and i == 2,
                                            perf_mode=DR)
                                else:
                                    for i, (xc, wc) in enumerate(
                                            ((0, 0), (1, 0), (0, 1))):
                                        nc.tensor.matmul(
                                            ps[:, 0:256],
                                            xt_pair(sh, p2, xc, msl),
                                            wvv[:, ks, wc, :],
                                            start=st and i == 0,
                                            stop=sp and i == 2,
                                            perf_mode=DR)
                            if g == 3:
                                mule = nc.gpsimd if upool is psBG else nc.vector
                                if kind == "q":
                                    rope_q(m, ps, csl, mule)
                                elif kind == "k":
                                    rope_k(m, ps, csl, mule)
                                else:
                                    nc.vector.tensor_scalar_mul(
                                        v_sb[sh * 4 + m][:], ps[:, 0:256],
                                        2.0 ** -8)
                        return f
                    out.extend(mmgrp(g) for g in range(4))
                return out

            def proj_direct(sh):
                for u, th in enumerate(proj_tiles(sh, [psS, psBG])):
                    th()

            bg_queue = []

            def enqueue_proj(sh):
                if (sh, 0) not in xt_tiles:
                    load_x(sh)
                for th in proj_tiles(sh, psBG):
                    bg_queue.append((f"proj{sh}", th))

            def oproj_m(m, pool, copy_eng):
                def nblk(n):
                    def f():
                        osb = outp.tile([128, 512], BF16, tag="ob",
                                        name="osb")
                        up = pool[n % 2] if isinstance(pool, list) else pool
                        if up is psBG:
                            ps = up.tile([128, 512], F32, tag="bg", name="pc")
                        else:
                            ps = up.tile([128, 1024], F32, tag="s", name="pc")
                        psv = ps[:, 0:512]
                        msl = slice(m * 128, m * 128 + 128)
                        nsl = slice(n * 512, n * 512 + 512)
                        for pr in range(2):
                            ksl = slice(2 * pr, 2 * pr + 2)
                            for i, (oc, wc) in enumerate(
                                    ((0, 0), (1, 0), (0, 1))):
                                nc.tensor.matmul(
                                    psv,
                                    otf8v[:, oc, ksl, msl],
                                    wov[:, ksl, wc, nsl],
                                    start=(pr == 0 and i == 0),
                                    stop=(pr == 1 and i == 2),
                                    perf_mode=DR,
                                )
                        ce = copy_eng[n % len(copy_eng)]
                        if ce == "act":
                            nc.scalar.copy(osb[:], psv)
                        elif ce == "pool":
                            nc.gpsimd.tensor_copy(osb[:], psv)
                        else:
                            nc.vector.tensor_copy(osb[:], psv)
                        nc.sync.dma_start(
                            out=out_d[m * 128:m * 128 + 128,
                                      n * 512:n * 512 + 512],
                            in_=osb[:])
                    return f
                return [nblk(n) for n in range(4)]

            def enqueue_oproj(g):
                for m in range(4 * g, 4 * g + 4):
                    for th in oproj_m(m, psBG, ("dve", "dve")):
                        bg_queue.append((f"oproj{g}", th))

            def oproj_direct(g):
                for m in range(4 * g, 4 * g + 4):
                    for th in oproj_m(m, [psS, psBG], ("act", "dve")):
                        th()

            def pump(n):
                for _ in range(n):
                    if not bg_queue:
                        return
                    bg_queue.pop(0)[1]()

            def flush_tag(tag):
                rest = []
                for t, th in bg_queue:
                    if t == tag:
                        th()
                    else:
                        rest.append((t, th))
                bg_queue[:] = rest

            def flush_all_bg():
                while bg_queue:
                    bg_queue.pop(0)[1]()

            # ============ attention ============
            def emit_ki_loop(qi, p):
                vh = p // 2
                q0 = 512 * qi
                kis = list(range(4 * qi, 4 * qi + 4)) + list(range(4 * qi))
                otbox = {}
                half = (qi * 4 + p) % 2
                smalls = smalls_all[:, half * 256:half * 256 + 128]
                nc.vector.memset(smalls[:], 0.0)
                nki = len(kis)

                def consume(idx, ki, ETv):
                    if "v" not in otbox:
                        OT = psOT.tile([128, 1024], F32, tag="ot", name="OT")
                        otbox["v"] = OT[:].rearrange("p (two n) -> p two n",
                                                     two=2)
                    OTv = otbox["v"]
                    j = ki - 4 * qi
                    diag = j >= 0
                    vc = 128 * j if diag and j > 0 else 0
                    vt = v_sb[ki][:, vh * 128:vh * 128 + 128]
                    st = idx == 0
                    sp = idx == nki - 1
                    for h in range(2):
                        nc.tensor.matmul(OTv[:, h, vc:512], vt,
                                         ETv[:, h, vc:512], start=st, stop=sp)
                    for h in range(2):
                        for c in range(4):
                            if diag and c < j:
                                continue
                            col = h * 64 + c * 16 + idx
                            nc.tensor.matmul(
                                smalls[:, col:col + 1],
                                ETv[:, h, c * 128:c * 128 + 128],
                                onescol[:],
                                start=True, stop=True,
                            )

                pend = []
                for idx, ki in enumerate(kis):
                    j = ki - 4 * qi
                    diag = j >= 0
                    vc = 128 * j if diag and j > 0 else 0
                    ksl = slice(ki * 128, ki * 128 + 128)
                    S12 = psS.tile([128, 1024], F32, tag="s", name="S12")
                    S12v = S12[:].rearrange("p (two n) -> p two n", two=2)
                    for h in range(2):
                        hsl = slice(h * 64, h * 64 + 64)
                        nc.tensor.matmul(
                            S12v[:, h, vc:512],
                            kTd[p][hsl, ksl],
                            qT_sb[p][hsl, q0 + vc:q0 + 512],
                            start=True, stop=True,
                        )
                    ET = etp.tile([128, 1024], BF16, tag="e", name="ET")
                    ETv = ET[:].rearrange("p (two n) -> p two n", two=2)
                    nc.scalar.activation(ETv[:, :, vc:512], S12v[:, :, vc:512],
                                         Act.Exp, scale=0.125)
                    if diag:
                        # zero the upper triangle of the local window on DVE
                        # (runs in the 2-ki consume-lag slack)
                        nc.gpsimd.tensor_mul(
                            ETv[:, :, vc:vc + 128], ETv[:, :, vc:vc + 128],
                            maskT[:].unsqueeze(1).broadcast_to([128, 2, 128]))
                    pend.append((idx, ki, ETv))
                    if len(pend) > 2:
                        consume(*pend.pop(0))
                    if idx < nki - 2 and (qi < 3 or idx % 2 == 0):
                        pump(1)
                for pc in pend:
                    consume(*pc)
                    pump(1)
                OTv = otbox["v"]

                OTs = ebp.tile([128, 1024], F32, tag="ots", name="OTs")
                nc.scalar.copy(OTs[:, 0:512], OTv[:, 0, :])
                nc.scalar.copy(OTs[:, 512:1024], OTv[:, 1, :])
                Rred = ebp.tile([128, 8], F32, tag="rred", name="Rred")
                nc.vector.tensor_reduce(
                    Rred[:],
                    smalls.rearrange("p (hc k) -> p hc k", k=16),
                    mybir.AxisListType.X, Alu.add)
                rcp2 = ebp.tile([128, 4], F32, tag="rcp", name="rcp2")
                nc.vector.reciprocal(rcp2[:], Rred[:, 4:8])
                m_q = ebp.tile([128, 4], F32, tag="mq", name="m_q")
                nc.vector.scalar_tensor_tensor(
                    m_q[:], Rred[:, 0:4], lam[:, 0:1], rcp2[:],
                    Alu.mult, Alu.mult)
                t2 = ebp.tile([128, 4], F32, tag="t2", name="t2")
                nc.vector.scalar_tensor_tensor(
                    t2[:], Rred[:, 0:4], SQ128E, Rred[:, 0:4],
                    Alu.mult, Alu.mult)
                return (qi, p, OTs, m_q, t2)

            def emit_late_epilogue(ctx):
                qi, p, OTs, m_q, t2 = ctx
                q0 = 512 * qi
                m_b = psS.tile([128, 1024], F32, tag="s", name="m_b")
                nc.tensor.transpose(m_b[0:4, 640:768], m_q[:], ident[:])
                mrow = ebp.tile([4, 128], BF16, tag="mrow", name="mrow")
                nc.vector.tensor_copy(mrow[:], m_b[0:4, 640:768])
                for c in range(4):
                    nc.tensor.matmul(m_b[:, c * 128:c * 128 + 128],
                                     sel4[:, c * 128:c * 128 + 128], mrow[:],
                                     start=True, stop=True)
                tt = ebp.tile([128, 512], F32, tag="tt", name="tt")
                nc.vector.tensor_mul(tt[:], OTs[:, 512:1024], m_b[:, 0:512])
                nc.gpsimd.tensor_sub(otf[p][:, q0:q0 + 512], OTs[:, 0:512],
                                     tt[:])
                sq = ebp.tile([128, 512], BF16, tag="sq", name="sq")
                nc.vector.tensor_mul(sq[:], otf[p][:, q0:q0 + 512],
                                     otf[p][:, q0:q0 + 512])
                for c in range(4):
                    nc.tensor.matmul(m_b[:, 512 + c:513 + c],
                                     sq[:, c * 128:c * 128 + 128],
                                     onescol[:], start=True, stop=True)
                off = (qi * 4 + p) * 4
                nc.vector.tensor_add(pre_all[:, off:off + 4],
                                     m_b[:, 512:516], t2[:])

            pending = [None]

            def attn_group(qi, post_first=None):
                for p in range(4):
                    ctx = emit_ki_loop(qi, p)
                    if pending[0] is not None:
                        emit_late_epilogue(pending[0])
                    pending[0] = ctx
                    if p == 0 and post_first is not None:
                        post_first()
                    pump(3)

            def flush_pending():
                if pending[0] is not None:
                    emit_late_epilogue(pending[0])
                    pending[0] = None

            def bfive_sf(qi):
                # sf = 4*(pre/128)^-0.5 = rsqrt(pre * 2^-11), via bit-hack +
                # 2 Newton steps on DVE (no act-table switch away from Exp).
                # The x4 spans otf8's fp8 hi/lo over a healthy e4m3 range;
                # host descales the output by 2^-8 total.
                goff = qi * 16
                gsl = slice(goff, goff + 16)
                I32 = mybir.dt.int32
                xt_ = ebp.tile([128, 16], F32, tag="lnp", name="nx")
                yt = ebp.tile([128, 16], F32, tag="ny", name="ny")
                t1 = ebp.tile([128, 16], F32, tag="nt", name="nt")
                nc.vector.tensor_scalar_mul(xt_[:], pre_all[:, gsl], 2.0 ** -11)
                nc.vector.tensor_scalar(
                    yt[:].bitcast(I32), xt_[:].bitcast(I32), 1, None,
                    Alu.logical_shift_right)
                nc.vector.tensor_scalar(
                    yt[:].bitcast(I32), yt[:].bitcast(I32), 0x5F3759DF, -1,
                    Alu.subtract, Alu.mult)
                for it in range(2):
                    dst = yt[:] if it == 0 else sf_all[:, gsl]
                    nc.vector.tensor_mul(t1[:], yt[:], yt[:])
                    nc.vector.tensor_mul(t1[:], t1[:], xt_[:])
                    nc.vector.tensor_scalar(t1[:], t1[:], -0.5, 1.5,
                                            Alu.mult, Alu.add)
                    nc.vector.tensor_mul(dst, yt[:], t1[:])

            def otf_store(p, q0, sfb):
                # otf8 hi = fp8(otf*sf) on DVE; Pool recomputes the bf16
                # product and subtracts hi for the lo residual
                qsl = slice(q0, q0 + 512)
                tn = ebp.tile([128, 512], BF16, tag="tn8", name="tn")
                nc.vector.tensor_mul(otf8v[:, 0, p, qsl],
                                     otf[p][:, qsl], sfb)
                nc.vector.tensor_mul(tn[:], otf[p][:, qsl], sfb)
                nc.gpsimd.tensor_sub(otf8v[:, 1, p, qsl], tn[:],
                                     otf8v[:, 0, p, qsl])

            def bfive_apply_one(qi, p, pool=None):
                goff = qi * 16
                q0 = 512 * qi
                off = goff + p * 4
                sf_b = psBG.tile([128, 512], F32, tag="bg", name="sf_b")
                nc.tensor.transpose(sf_b[0:4, 0:128],
                                    sf_all[:, off:off + 4], ident[:])
                sfrow = ebp.tile([4, 128], BF16, tag="mrow", name="sfrow")
                nc.vector.tensor_copy(sfrow[:], sf_b[0:4, 0:128])
                for c in range(4):
                    nc.tensor.matmul(sf_b[:, c * 128:c * 128 + 128],
                                     sel4[:, c * 128:c * 128 + 128], sfrow[:],
                                     start=True, stop=True)
                otf_store(p, q0, sf_b[:, 0:512])

            def enqueue_bfive_apply(qi):
                for p in range(4):
                    bg_queue.append((f"bfa{qi}",
                                     (lambda qq, pp: lambda: bfive_apply_one(qq, pp))(qi, p)))

            def bfive_group(qi, filler=()):
                # applies interleaved two-way so the transpose/copy/broadcast
                # chains of consecutive heads overlap through the psS ring;
                # filler thunks (independent oproj blocks) keep PE fed while
                # the DVE/Pool chains drain
                filler = list(filler)
                bfive_sf(qi)
                goff = qi * 16
                q0 = 512 * qi
                sfbs = {}
                rows = {}
                for ph in range(0, 4, 2):
                    for p in (ph, ph + 1):
                        off = goff + p * 4
                        sfbs[p] = psS.tile([128, 1024], F32, tag="s",
                                           name="sf_b")
                        nc.tensor.transpose(sfbs[p][0:4, 640:768],
                                            sf_all[:, off:off + 4], ident[:])
                    for p in (ph, ph + 1):
                        rows[p] = ebp.tile([4, 128], BF16, tag="mrow",
                                           name="sfrow")
                        nc.vector.tensor_copy(rows[p][:], sfbs[p][0:4, 640:768])
                    for _ in range(2):
                        if filler:
                            filler.pop(0)()
                    for p in (ph, ph + 1):
                        for c in range(4):
                            nc.tensor.matmul(
                                sfbs[p][:, c * 128:c * 128 + 128],
                                sel4[:, c * 128:c * 128 + 128], rows[p][:],
                                start=True, stop=True)
                    for p in (ph, ph + 1):
                        otf_store(p, q0, sfbs[p][:, 0:512])
                    for _ in range(2):
                        if filler:
                            filler.pop(0)()
                for th in filler:
                    th()

            # ============ schedule ============
            def aftern1():
                bfive_sf(0)
                enqueue_bfive_apply(0)

            def after0():
                enqueue_oproj(0)
                bfive_sf(1)
                enqueue_bfive_apply(1)

            def after1():
                enqueue_oproj(1)
                bfive_sf(2)
                enqueue_bfive_apply(2)
                for m in (8, 9):
                    for th in oproj_m(m, psBG, ("dve", "dve")):
                        bg_queue.append(("oproj2", th))

            proj_direct(0)
            proj_direct(1)
            enqueue_proj(2)
            attn_group(0)
            enqueue_proj(3)
            attn_group(1, post_first=aftern1)
            flush_tag("proj2")
            attn_group(2, post_first=after0)
            flush_tag("proj3")
            attn_group(3, post_first=after1)
            flush_all_bg()
            flush_pending()
            fill = [th for m in (10, 11)
                    for th in oproj_m(m, psBG, ("act", "dve"))]
            bfive_group(3, filler=fill)
            oproj_direct(3)

    nc.compile()
    return nc


def _perm64():
    return np.array(list(range(0, 16)) + list(range(32, 48)) +
                    list(range(16, 32)) + list(range(48, 64)))


def _host_tables():
    p64 = _perm64()
    inv = ROPE_THETA ** (-np.arange(Dh, dtype=np.float64) / Dh)
    pos = np.arange(S, dtype=np.float64)
    fr = pos[:, None] * inv[None, :]              # [S, 64]
    cos = np.cos(fr).astype(np.float32)           # [S, 64]
    sin = np.sin(fr).astype(np.float32)
    d = p64[np.arange(128) % 64]
    # 2^-8 descales the fp8 weight (x64) and x (x4) scales out of the q/k psum
    cosT = (np.ascontiguousarray(cos[:, d].T)
            * np.float32(2.0 ** -8)).astype(ml_dtypes.bfloat16)
    sgn = np.where(d < 32, -1.0, 1.0).astype(np.float32)
    ssinT = (np.ascontiguousarray(sin[:, d].T * sgn[:, None])
             * np.float32(2.0 ** -8)).astype(ml_dtypes.bfloat16)
    # scores psum carries 8*s_true (exp applies scale=1/8); mask -50 -> -400
    # E rows are k-positions, cols are q: keep kp <= q
    maskT = np.triu(np.ones((128, 128), np.float32)).astype(
        ml_dtypes.bfloat16)
    ident = np.eye(128, dtype=np.float32)
    onescol = np.ones((128, 1), np.float32).astype(ml_dtypes.bfloat16)
    sel4 = np.zeros((4, 512), np.float32)
    for c in range(4):
        sel4[c, c * 128:(c + 1) * 128] = 1.0
    sel4 = sel4.astype(ml_dtypes.bfloat16)
    return cosT, ssinT, maskT, ident, onescol, sel4


def kernel(hidden_states, Wq, Wk, Wv, Wo,
           lambda_q1, lambda_k1, lambda_q2, lambda_k2, subln_weight):
    from concourse.bass_utils import run_bass_kernel_spmd

    if "nc" not in _CACHE:
        _CACHE["nc"] = _build_nc()
        _CACHE["tables"] = _host_tables()
    nc = _CACHE["nc"]
    cosT, ssinT, maskT, ident, onescol, sel4 = _CACHE["tables"]

    f32 = np.float32
    bf16 = ml_dtypes.bfloat16
    hs = np.asarray(hidden_states, f32)
    Wq = np.asarray(Wq, f32)
    Wk = np.asarray(Wk, f32)
    Wv = np.asarray(Wv, f32)
    Wo = np.asarray(Wo, f32)
    subln = np.asarray(subln_weight, f32)

    lam1 = np.exp(np.sum(np.asarray(lambda_q1, f32) * np.asarray(lambda_k1, f32),
                         dtype=f32))
    lam2 = np.exp(np.sum(np.asarray(lambda_q2, f32) * np.asarray(lambda_k2, f32),
                         dtype=f32))
    lam_full = f32(lam1 - lam2 + LAMBDA_INIT)
    lam_arr = np.full((128, 1), lam_full, f32)

    wprime = (np.tile(subln, H) * f32(1.0 - LAMBDA_INIT)).astype(f32)  # [2048]
    WoS = Wo * wprime[None, :]

    p64 = _perm64()
    qperm = (np.repeat(np.arange(8) * 64, 64) + np.tile(p64, 8))
    kperm = (np.repeat(np.arange(4) * 64, 64) + np.tile(p64, 4))

    f8 = ml_dtypes.float8_e4m3

    def hilo(a):
        hi = np.ascontiguousarray(a).astype(f8)
        lo = (a - hi.astype(f32)).astype(f8)
        return np.ascontiguousarray(np.concatenate([hi, lo], axis=1))

    x8 = {b: hilo(hs[b].T * f32(4.0)) for b in range(B)}

    in_maps = []
    for c in range(NCORES):
        b, r = c // TP, c % TP
        wq_h = hilo(Wq[512 * r:512 * r + 512, :].T[:, qperm] * f32(64.0))
        wk_h = hilo(Wk[256 * r:256 * r + 256, :].T[:, kperm] * f32(64.0))
        wv_h = hilo(Wv[256 * r:256 * r + 256, :].T * f32(64.0))
        wo_h = hilo(WoS[:, 512 * r:512 * r + 512].T * f32(64.0))
        in_maps.append({
            "xT": x8[b],
            "wqT": wq_h, "wkT": wk_h, "wvT": wv_h, "woT": wo_h,
            "cosT": cosT, "ssinT": ssinT, "maskT": maskT,
            "ident": ident, "onescol": onescol, "sel4": sel4,
            "lam": lam_arr,
        })

    res = run_bass_kernel_spmd(nc, in_maps, core_ids=list(range(NCORES)))
    out = np.zeros((B, S, D), f32)
    for c in range(NCORES):
        out[c // TP] += np.asarray(res.results[c]["out"]).astype(f32)
    # descale the fp8 wo (x64) and otf (x4 via the sf bias) scales
    out *= f32(2.0 ** -8)
    return out

